# revision 8
# baseline (speedup 1.0000x reference)
"""Trainium2 Bass kernel for the e3nn-style concat + per-irrep Linear problem.

Reference computation (N = 200000 nodes, 480-dim features per input):
  per input: 128x0e (dims 0:128) + 64x1e (dims 128:320) + 32x2e (dims 320:480)
  s = [s1, s2] @ W0 * inv0 + b0                   # [N, 128]
  v = einsum('nmi,mo->noi', [v1,v2], W1) * inv1   # [N, 64, 3]
  t = einsum('nmi,mo->noi', [t1,t2], W2) * inv2   # [N, 32, 5]
  out = concat([s, v.flat, t.flat], axis=1)       # [N, 480]

Strategy (memory-bound, data-parallel over nodes across 8 cores):
  - Everything streams as bf16 (quantization error ~3e-3 vs the 2e-2
    gate) to halve HBM traffic vs fp32.
  - DRAM layout is partition-major per 1000-node block: each SBUF
    partition's slice of a block is one contiguous 7-8 KB run in DRAM.
  - 23 uniform blocks of 1000 nodes + 4 tail half-blocks of 500 that
    reuse the SAME tile shapes (DMA into a slice) so the tile-pool ring
    never changes shape (a shape change flushes the pipeline, ~20us).
  - The small t4 slab ([64, NB]) is folded into the xinb tile as a
    [128, NB/2] region (partition p carries t4[p%64, (p//64)*NB/2 + j]),
    so each block is exactly 2 loads (1.0 MB + 0.875 MB) + 1 store.
    The two loads alternate between the two HWDGE rings per block.
  - PE work is trimmed to 15 moving passes per block (the tensor engine
    p-state ramp makes matmuls ~2x slower after any idle gap, so PE
    must stay well under the DMA budget):
      * both 500-col halves of t4 are computed in ONE K=128 pass with
        blockdiag(W2s, W2s): the h=1 result lands in chunk-3 rows
        96:128 of the h=0 slice (previously junk), recovered on host;
      * bias is folded into the DVE drain (tensor_scalar_add of a
        [128,1] fp32 bias) instead of a K=1 ones-vector matmul.
  - Per 500-col slice the four output row chunks land in one 4-bank
    PSUM tile; two DVE instructions drain it (chunk0 with bias add,
    chunks 1-3 strided copy), one SWDGE store per block.
  - Host: transpose/interleave the bf16 out tensor back to the fp32
    reference layout.
"""

import numpy as np
import ml_dtypes

BF16 = ml_dtypes.bfloat16
MUL0, MUL1, MUL2 = 128, 64, 32
N_TOTAL = 200000
N_CORES = 8
NC_NODES = N_TOTAL // N_CORES          # 25000
NODE_BLOCK = 1000
HALF = NODE_BLOCK // 2                 # 500-col matmul chunks (fp32 PSUM bank)
QTR = HALF // 2                        # 250: folded-t4 width of a half block
N_BLOCKS = 23                          # full 1000-node blocks
N_TBLOCKS = 4                          # tail half-blocks of HALF nodes
MAIN_NODES = N_BLOCKS * NODE_BLOCK     # 23000
NPAD = NC_NODES

_PROGRAM_CACHE = {}


def _build_program():
    import concourse.mybir as mybir
    from concourse import bacc
    import concourse.tile as tile

    f32 = mybir.dt.float32
    bf16 = mybir.dt.bfloat16
    NB = NODE_BLOCK
    nc = bacc.Bacc("TRN2", target_bir_lowering=False, debug=False)

    nblocks = N_BLOCKS
    # Partition-major block layouts: row p of block b is p's whole SBUF
    # line, so each DMA descriptor is one contiguous 7-8 KB run.
    # xina: slabs [s1|s2|v0|v1]; xinb: [v2|t01|t23|t4fold] where t4fold is
    # the [64, NB] t4 slab refolded to [128, NB/2].
    xina = nc.dram_tensor("xina", [nblocks, 128, 4 * NB], bf16, kind="ExternalInput").ap()
    xinb = nc.dram_tensor("xinb", [nblocks, 128, 3 * NB + HALF], bf16, kind="ExternalInput").ap()
    xina2 = nc.dram_tensor("xina2", [N_TBLOCKS, 128, 4 * HALF], bf16, kind="ExternalInput").ap()
    xinb2 = nc.dram_tensor("xinb2", [N_TBLOCKS, 128, 3 * HALF + QTR], bf16, kind="ExternalInput").ap()
    w0a = nc.dram_tensor("w0a", [128, 128], bf16, kind="ExternalInput").ap()
    w0b = nc.dram_tensor("w0b", [128, 128], bf16, kind="ExternalInput").ap()
    w1d = nc.dram_tensor("w1d", [128, 64], bf16, kind="ExternalInput").ap()
    w2p = nc.dram_tensor("w2p", [128, 64], bf16, kind="ExternalInput").ap()
    b0f = nc.dram_tensor("b0f", [128, 1], f32, kind="ExternalInput").ap()
    # Store tensors; chunk-3 rows 96:128 of each h=0 slice carry the h=1
    # t4 output (from the combined blockdiag pass); the h=1 slice's rows
    # 64:128 of chunk 3 are junk the host drops.
    outa = nc.dram_tensor("outa", [nblocks, 128, 4 * NB], bf16, kind="ExternalOutput").ap()
    outa2 = nc.dram_tensor("outa2", [N_TBLOCKS, 128, 4 * HALF], bf16, kind="ExternalOutput").ap()

    with tile.TileContext(nc) as tc:
        with (
            tc.tile_pool(name="wpool", bufs=1) as wpool,
            tc.tile_pool(name="inpool", bufs=4) as inpool,
            tc.tile_pool(name="psum", bufs=2, space="PSUM") as psum,
            tc.tile_pool(name="outpool", bufs=4) as outpool,
        ):
            wa_t = wpool.tile([128, 128], bf16)
            wb_t = wpool.tile([128, 128], bf16)
            w1_t = wpool.tile([128, 64], bf16)
            w2p_t = wpool.tile([128, 64], bf16)
            b0_t = wpool.tile([128, 1], f32)
            # Weights ride the SWDGE ring so the HWDGE rings start streaming
            # block loads immediately.
            nc.gpsimd.dma_start(wa_t[:], w0a)
            nc.gpsimd.dma_start(wb_t[:], w0b)
            nc.gpsimd.dma_start(w1_t[:], w1d)
            nc.gpsimd.dma_start(w2p_t[:], w2p)
            nc.gpsimd.dma_start(b0_t[:], b0f)

            for blk in range(nblocks):
                # Two loads per block, alternating HWDGE rings so both rings
                # carry ~the same bytes. Stores go through SWDGE (gpsimd).
                tina = inpool.tile([128, 4 * NB], bf16)
                tinb = inpool.tile([128, 3 * NB + HALF], bf16)
                qa, qb = (nc.sync, nc.scalar) if blk % 2 == 0 else (nc.scalar, nc.sync)
                qa.dma_start(tina[:], xina[blk])
                qb.dma_start(tinb[:], xinb[blk])

                # out row chunks: [s(128)] [v0|v1] [v2|t0,t1] [t2,t3|t4(h0);t4(h1)]
                tout = outpool.tile([128, 4 * NB], bf16)
                for h in range(2):
                    o = h * HALF
                    pbig = psum.tile([128, 2048], f32)
                    nc.tensor.matmul(pbig[:, 0:HALF], wa_t[:], tina[:, o:o + HALF],
                                     start=True, stop=False)
                    nc.tensor.matmul(pbig[:, 0:HALF], wb_t[:], tina[:, NB + o:NB + o + HALF],
                                     start=False, stop=True)
                    nc.tensor.matmul(pbig[0:64, 512:512 + HALF], w1_t[:], tina[:, 2 * NB + o:2 * NB + o + HALF])
                    nc.tensor.matmul(pbig[64:128, 512:512 + HALF], w1_t[:], tina[:, 3 * NB + o:3 * NB + o + HALF])
                    nc.tensor.matmul(pbig[0:64, 1024:1024 + HALF], w1_t[:], tinb[:, o:o + HALF])
                    nc.tensor.matmul(pbig[64:128, 1024:1024 + HALF], w2p_t[:], tinb[:, NB + o:NB + o + HALF])
                    nc.tensor.matmul(pbig[0:64, 1536:1536 + HALF], w2p_t[:], tinb[:, 2 * NB + o:2 * NB + o + HALF])
                    if h == 0:
                        # both halves' t4 in one pass; h=1 half lands in
                        # rows 96:128 (recovered host-side)
                        nc.tensor.matmul(pbig[64:128, 1536:1536 + HALF], w2p_t[:],
                                         tinb[:, 3 * NB:3 * NB + HALF])

                    nc.vector.tensor_scalar_add(tout[:, o:o + HALF], pbig[:, 0:HALF], b0_t[:])
                    nc.vector.tensor_copy(
                        tout[:].rearrange("p (c n) -> p c n", n=NB)[:, 1:4, o:o + HALF],
                        pbig[:].rearrange("p (c n) -> p c n", n=512)[:, 1:4, 0:HALF],
                    )

                nc.gpsimd.dma_start(outa[blk], tout[:])

            # Tail: 4 half-blocks with IDENTICAL tile shapes (loads/stores hit
            # slices) so the pool rings stay uniform and nothing flushes.
            for u in range(N_TBLOCKS):
                tina = inpool.tile([128, 4 * NB], bf16)
                tinb = inpool.tile([128, 3 * NB + HALF], bf16)
                qa, qb = (nc.sync, nc.scalar) if u % 2 == 0 else (nc.scalar, nc.sync)
                qa.dma_start(tina[:, 0:4 * HALF], xina2[u])
                qb.dma_start(tinb[:, 0:3 * HALF + QTR], xinb2[u])

                tout = outpool.tile([128, 4 * NB], bf16)
                pbig = psum.tile([128, 2048], f32)
                nc.tensor.matmul(pbig[:, 0:HALF], wa_t[:], tina[:, 0:HALF],
                                 start=True, stop=False)
                nc.tensor.matmul(pbig[:, 0:HALF], wb_t[:], tina[:, HALF:2 * HALF],
                                 start=False, stop=True)
                nc.tensor.matmul(pbig[0:64, 512:512 + HALF], w1_t[:], tina[:, 2 * HALF:3 * HALF])
                nc.tensor.matmul(pbig[64:128, 512:512 + HALF], w1_t[:], tina[:, 3 * HALF:4 * HALF])
                nc.tensor.matmul(pbig[0:64, 1024:1024 + HALF], w1_t[:], tinb[:, 0:HALF])
                nc.tensor.matmul(pbig[64:128, 1024:1024 + HALF], w2p_t[:], tinb[:, HALF:2 * HALF])
                nc.tensor.matmul(pbig[0:64, 1536:1536 + HALF], w2p_t[:], tinb[:, 2 * HALF:3 * HALF])
                # combined t4: 250 cols; second 250 nodes land in rows 96:128
                nc.tensor.matmul(pbig[64:128, 1536:1536 + QTR], w2p_t[:],
                                 tinb[:, 3 * HALF:3 * HALF + QTR])

                nc.vector.tensor_scalar_add(tout[:, 0:HALF], pbig[:, 0:HALF], b0_t[:])
                nc.vector.tensor_copy(
                    tout[:].rearrange("p (c n) -> p c n", n=HALF)[:, 1:4, 0:HALF],
                    pbig[:].rearrange("p (c n) -> p c n", n=512)[:, 1:4, 0:HALF],
                )
                nc.gpsimd.dma_start(outa2[u], tout[:, 0:4 * HALF])

    nc.compile()
    return nc


def _get_program(key="bf16"):
    key = "bf16"
    if key not in _PROGRAM_CACHE:
        _PROGRAM_CACHE[key] = _build_program()
    return _PROGRAM_CACHE[key]


def _repack_inputs(x1, x2):
    """Build the eight 128-row contraction slabs [960, N] in bf16.

    Slabs: [s1] [s2] [v1_0|v2_0] [v1_1|v2_1] [v1_2|v2_2]
    [t_0|t_1] [t_2|t_3] [t_4], each t_i = [t1_i(32); t2_i(32)].
    """
    n = x1.shape[0]
    x1b = x1.astype(BF16)
    x2b = x2.astype(BF16)
    xr = np.empty((960, n), dtype=BF16)
    xr[0:128] = x1b[:, 0:128].T
    xr[128:256] = x2b[:, 0:128].T
    v1 = x1b[:, 128:320].reshape(n, MUL1, 3)
    v2 = x2b[:, 128:320].reshape(n, MUL1, 3)
    for i in range(3):
        base = 256 + 128 * i
        xr[base:base + 64] = v1[:, :, i].T
        xr[base + 64:base + 128] = v2[:, :, i].T
    t1 = x1b[:, 320:480].reshape(n, MUL2, 5)
    t2 = x2b[:, 320:480].reshape(n, MUL2, 5)
    for i in range(5):
        base = 640 + 64 * i
        xr[base:base + 32] = t1[:, :, i].T
        xr[base + 32:base + 64] = t2[:, :, i].T
    return xr


def _to_pmajor(sl, nrows, nchunks, nblocks, nb):
    """[nchunks*nrows, nblocks*nb] slab-major -> [nblocks, nrows, nchunks*nb] p-major."""
    # sl[c*nrows + p, b*nb + j] -> out[b, p, c*nb + j]
    return np.ascontiguousarray(
        sl.reshape(nchunks, nrows, nblocks, nb).transpose(2, 1, 0, 3)
        .reshape(nblocks, nrows, nchunks * nb)
    )


def _fold_t4(t4, nblocks, nb):
    """t4 [64, nblocks*nb] -> [nblocks, 128, nb/2]: partition p of block b
    carries t4[p%64, b*nb + (p//64)*(nb/2) + j]."""
    h = nb // 2
    t4 = t4.reshape(64, nblocks, nb)
    out = np.empty((nblocks, 128, h), dtype=t4.dtype)
    out[:, 0:64, :] = t4[:, :, 0:h].transpose(1, 0, 2)
    out[:, 64:128, :] = t4[:, :, h:].transpose(1, 0, 2)
    return out


def _prepare_in_maps(x1, x2, W0, W1, W2, b0):
    x1 = np.asarray(x1, dtype=np.float32)
    x2 = np.asarray(x2, dtype=np.float32)
    inv0 = np.float32(1.0 / np.sqrt(2 * MUL0))
    inv1 = np.float32(1.0 / np.sqrt(2 * MUL1))
    inv2 = np.float32(1.0 / np.sqrt(2 * MUL2))
    w0s = np.asarray(W0, np.float32) * inv0                            # [256, 128]
    w1s = np.asarray(W1, np.float32) * inv1                            # [128, 64]
    w2s = np.asarray(W2, np.float32) * inv2                            # [64, 32]
    w2pair = np.zeros((128, 64), dtype=np.float32)                     # blockdiag(W2s, W2s)
    w2pair[0:64, 0:32] = w2s
    w2pair[64:128, 32:64] = w2s
    weights = {
        "w0a": np.ascontiguousarray(w0s[0:128]).astype(BF16),
        "w0b": np.ascontiguousarray(w0s[128:256]).astype(BF16),
        "w1d": np.ascontiguousarray(w1s).astype(BF16),
        "w2p": w2pair.astype(BF16),
        "b0f": np.asarray(b0, np.float32).reshape(128, 1),
    }
    xr = _repack_inputs(x1, x2)
    NB = NODE_BLOCK
    in_maps = []
    for c in range(N_CORES):
        xc = xr[:, c * NC_NODES:(c + 1) * NC_NODES]
        xm, xt = xc[:, 0:MAIN_NODES], xc[:, MAIN_NODES:]
        xinb = np.empty((N_BLOCKS, 128, 3 * NB + HALF), dtype=BF16)
        xinb[:, :, 0:3 * NB] = _to_pmajor(xm[512:896], 128, 3, N_BLOCKS, NB)
        xinb[:, :, 3 * NB:] = _fold_t4(xm[896:960], N_BLOCKS, NB)
        xinb2 = np.empty((N_TBLOCKS, 128, 3 * HALF + QTR), dtype=BF16)
        xinb2[:, :, 0:3 * HALF] = _to_pmajor(xt[512:896], 128, 3, N_TBLOCKS, HALF)
        xinb2[:, :, 3 * HALF:] = _fold_t4(xt[896:960], N_TBLOCKS, HALF)
        in_maps.append({
            "xina": _to_pmajor(xm[0:512], 128, 4, N_BLOCKS, NB),
            "xinb": xinb,
            "xina2": _to_pmajor(xt[0:512], 128, 4, N_TBLOCKS, HALF),
            "xinb2": xinb2,
            **weights,
        })
    return in_maps


def _from_pmajor(oa, nblocks, nb):
    """[nblocks,128,4*nb] p-major -> [480, nblocks*nb] slab-major.

    Chunks 0-2 rows 0:128 -> slab rows 0:384; chunk 3 rows 0:64 -> t2,t3
    (rows 384:448); t4 (rows 448:480) comes from chunk 3 of the FIRST half
    slice: rows 64:96 = nodes [0, nb/2), rows 96:128 = nodes [nb/2, nb).
    """
    h = nb // 2
    oa = np.asarray(oa).reshape(nblocks, 128, 4, nb)
    o3 = (oa[:, :, 0:3, :].transpose(2, 1, 0, 3)
          .reshape(3 * 128, nblocks * nb).astype(np.float32))
    t23 = (oa[:, 0:64, 3, :].transpose(1, 0, 2)
           .reshape(64, nblocks * nb).astype(np.float32))
    t4 = np.empty((32, nblocks, nb), dtype=np.float32)
    t4[:, :, 0:h] = oa[:, 64:96, 3, 0:h].transpose(1, 0, 2)
    t4[:, :, h:] = oa[:, 96:128, 3, 0:h].transpose(1, 0, 2)
    return np.concatenate([o3, t23, t4.reshape(32, nblocks * nb)], axis=0)


def _assemble_output(outs):
    """outs: list of 8 (outa [23,128,4000], outa2 [4,128,2000]) -> [N_TOTAL, 480]."""
    full = np.empty((N_TOTAL, 480), dtype=np.float32)
    for c, (oa, oa2) in enumerate(outs):
        o = np.concatenate([
            _from_pmajor(oa, N_BLOCKS, NODE_BLOCK),
            _from_pmajor(oa2, N_TBLOCKS, HALF),
        ], axis=1)                                 # [480, NC_NODES] slab-major
        rows = slice(c * NC_NODES, (c + 1) * NC_NODES)
        full[rows, 0:128] = o[0:128].T
        full[rows, 128:320] = (
            o[128:320].reshape(3, MUL1, NC_NODES).transpose(2, 1, 0).reshape(NC_NODES, 192)
        )
        full[rows, 320:480] = (
            o[320:480].reshape(5, MUL2, NC_NODES).transpose(2, 1, 0).reshape(NC_NODES, 160)
        )
    return full


def kernel(x1, x2, W0, W1, W2, b0):
    from concourse.bass_utils import run_bass_kernel_spmd

    in_maps = _prepare_in_maps(x1, x2, W0, W1, W2, b0)
    nc = _get_program()
    res = run_bass_kernel_spmd(nc, in_maps, core_ids=list(range(N_CORES)))
    return _assemble_output([(r["outa"], r["outa2"]) for r in res.results])
